# revision 58
# baseline (speedup 1.0000x reference)
"""Trainium2 Bass kernel for 16->16 channel 3x3 VALID conv on [16,1536,1536].

out[co, y, x] = sum_{ci,dy,dx} W[co,ci,dy,dx] * X[ci, y+dy, x+dx] + sum(bias)

Strategy (8-core data parallel over H, halo of 2 rows):
  Each core computes 192 output rows from a 194-row input shard, in 32 blocks
  of R=6 output rows. Per block, an SBUF "window" of 8 input rows x 16
  channels is laid out as [128, 1536] with partition p = ci*8 + k (k = row in
  window). The conv becomes 3 accumulating matmuls (one per kernel column dx)
  whose dx shift is a free-dim offset on the moving operand:
     psum[m=(co*6+r), x] += lhsT_dx[p, m] * window[p, x+dx]
  with block-Toeplitz weights lhsT_dx[ci*8+k, co*6+r] = W[co,ci,k-r,dx]
  (zero outside 0 <= k-r <= 2), precomputed on host from the 9KB weight.
  Contraction K=128, M=96 -> 3 column-streams per 6 output pixels; measured
  PE cadence is the N/2.4GHz streaming floor (216ns per 512-col matmul,
  LDWEIGHTS fully hidden), ~62us/core for the sweep.

  Matmuls run in float16 (1 col/cycle; fp32 would be 4x slower). Inputs are
  cast to f16 on host (halves input DMA bytes); outputs are stored f16 and
  upcast on host (halves output DMA bytes; tolerance is 2e-2, f16 adds
  ~4e-4). HBM/core: 12.6 MB in + 9.4 MB out = 22 MB ~= 61.5us at 358 GB/s,
  right at the PE floor ("ridge").

  Inputs arrive as host-preduped window groups of 4 (one contiguous DMA per
  group on the sync-engine HWDGE queue; the 33% halo duplication is cheaper
  than any on-chip halo-carry variant - those all lose to in-order engine /
  DMA-completion-semaphore serialization, measured 96-148us). Group 0 loads
  per-window tiles so the first matmul starts at ~11us. PSUM eviction (+bias,
  f32->f16 cast) is split between the scalar engine (chunk 0, and chunk 2 on
  even windows) and the vector engine (chunk 1, chunk 2 on odd windows).
  Outputs leave as one grouped DMA per 4 windows on the scalar-engine HWDGE
  queue; the last group stores per-window so the final drain is 1/4 the size.
  Weights load via the scalar queue so the first input DMA owns sync.
"""

import numpy as np

import concourse.bass as bass
import concourse.mybir as mybir
import concourse.tile as tile
from concourse.bass_utils import run_bass_kernel_spmd

C = 16
H = 1536
W = 1536
HOUT = H - 2
WOUT = W - 2
NCORES = 8
ROWS_PER_CORE = 192  # output rows computed per core
R = 6                # output rows per block
WIN = R + 2          # input rows per window
NBLK = ROWS_PER_CORE // R
XIN_ROWS = ROWS_PER_CORE + 2  # input rows per shard
CHUNKS = [(0, 512), (512, 512), (1024, WOUT - 1024)]
F32 = mybir.dt.float32
F32R = mybir.dt.float32r

_drain_patched = False
# (SyncWait, value) pairs whose wait_value is injected after the Tile
# scheduler's CoreSim pass (the sim cannot see pre-TileContext DMA
# completions, so the waits are emitted as >=0 and patched to the real
# thresholds just before lowering)
_PREGATE = []


def _patch_tile_drain():
    """This container's walrus accepts only ONE sync-wait per lowered
    instruction (CTRL drains, S3_LW weight loads, ...). Tile freely attaches
    several. Split the extras onto single-wait nops placed just before the
    instruction on the same engine (identical blocking semantics)."""
    global _drain_patched
    if _drain_patched:
        return
    _drain_patched = True
    from concourse.tile import ScopedClock

    def _split_multi_waits(ordered):
        for bb_name, insts in ordered.items():
            out = []
            for inst in insts:
                si = getattr(inst, "sync_info", None)
                if (
                    si is not None
                    and si.on_wait is not None
                    and len(si.on_wait) > 1
                    and type(inst).__module__ == "bass_rust"
                ):
                    waits = list(si.on_wait)
                    for i, w in enumerate(waits[:-1]):
                        out.append(
                            mybir.InstNoOp(
                                name=f"{inst.name}ws{i}",
                                engine=inst.engine,
                                bass_nofuse=True,
                                sync_info=mybir.SyncInfo(
                                    on_wait=[w], on_update=[]
                                ),
                            )
                        )
                    inst.sync_info = mybir.SyncInfo(
                        on_wait=[waits[-1]],
                        on_update=list(si.on_update or []),
                    )
                out.append(inst)
            ordered[bb_name] = out
        return ordered

    orig_lower = tile.TileContext._lower_ordered_insts

    def _lower_ordered_insts(self, ordered):
        for wobj, val in _PREGATE:
            wobj.wait_value = val
        _PREGATE.clear()
        return orig_lower(self, _split_multi_waits(ordered))

    tile.TileContext._lower_ordered_insts = _lower_ordered_insts

    def _drain_and_barrier(self, tick_clock, wait_clock):
        drain_inst = self.nc.sync.drain()
        wait_clock.add_sem_waits(
            drain_inst.ins, ScopedClock({None: tick_clock.global_clock})
        )
        si = drain_inst.ins.sync_info
        if si is not None and si.on_wait is not None and len(si.on_wait) > 1:
            waits = list(si.on_wait)
            drain_inst.ins.sync_info = mybir.SyncInfo(
                on_wait=[waits[0]], on_update=list(si.on_update or [])
            )
            for w in waits[1:]:
                n = self.nc.sync.nop(nofuse=True, hint="drain_wait_split")
                n.ins.sync_info = mybir.SyncInfo(on_wait=[w], on_update=[])
        self.nc.all_engine_barrier()
        assert self.sems is not None
        popped = self.nc._tile_sem_poison_stack.pop()
        assert popped is self._sem_poison
        self.nc.clear_and_free_semaphores(list(self.sems.allocated().values()))
        self.nc.all_engine_barrier()

    tile.TileContext._drain_and_barrier = _drain_and_barrier


def build_lhsT(weight: np.ndarray, k_major: bool = False) -> np.ndarray:
    """[C_out=16, C_in=16, 3, 3] -> [3, 128, 96] block-Toeplitz stationary
    operands, one per kernel column dx.
    ci-major: lhsT[dx, ci*8+k, co*6+r] = weight[co, ci, k-r, dx] (0<=k-r<=2)
    k-major:  lhsT[dx, k*16+ci, co*6+r] = same (used by the halo path, where
    window rows k must be partition-contiguous for the DVE halo copies)."""
    lhsT = np.zeros((3, 128, 96), np.float32)
    ci = np.arange(C)
    for dx in range(3):
        for dy in range(3):
            for r in range(R):
                k = r + dy
                p = (k * C + ci) if k_major else (ci * WIN + k)
                lhsT[dx, p[:, None], (np.arange(C) * R + r)[None, :]] = (
                    weight[:, :, dy, dx].T
                )
    return lhsT


def shard_rows(Xs: np.ndarray, dtype=np.float16):
    """Halo-path host prep for one core's shard [C, XIN_ROWS, W]: rows
    shipped once, row-outer [XIN_ROWS, C, W] so (k c) merges contiguously
    in the window DMAs."""
    return np.ascontiguousarray(Xs.transpose(1, 0, 2).astype(dtype))


def shard_windows(
    Xs: np.ndarray, group: int, dtype=np.float32
) -> np.ndarray:
    """Host-side window predup for one core's shard [C, XIN_ROWS, W] ->
    [ngroups, 128, group*W] where out[g, ci*8+k, w*W+x] =
    Xs[ci, 6*(group*g+w)+k, x]. dma_start carries a large fixed overhead
    here, so shipping the 33% halo duplication in exchange for one
    contiguous multi-MB DMA per group is a clear win. For 16-bit matmul
    dtypes the cast happens here too, halving the DMA bytes."""
    ngroups = NBLK // group
    rows = (
        R * group * np.arange(ngroups)[:, None, None]
        + R * np.arange(group)[None, :, None]
        + np.arange(WIN)[None, None, :]
    )  # [g, w, k]
    arr = Xs[:, rows, :].astype(dtype)  # [C, g, w, k, W]
    arr = arr.transpose(1, 0, 3, 2, 4)  # [g, C, k, w, W]
    return np.ascontiguousarray(
        arr.reshape(ngroups, 128, group * W)
    )


def build_program(
    bias_sum: float,
    mm_dtype=F32R,
    nblk=NBLK,
    group=4,
    xbufs=3,
    wbufs=2,
    obufs=4,
    pbufs=6,
    repeat=1,
    hw_loop=0,
    out16=True,
    split_evict=True,
    halo=True,
):
    """One core's program: see module docstring. dma_start carries a large
    fixed overhead in this environment, so inputs arrive as host-preduped
    window groups (one contiguous DMA per `group` blocks) and outputs leave
    as one grouped DMA into a [C, R, nblk, WOUT] device layout the host
    re-transposes. `repeat` (python-unrolled) and `hw_loop` (tc.For_i)
    re-run the whole block sweep, for timing amplification only."""
    nc = bass.Bass("TRN2", target_bir_lowering=False, debug=False)
    assert nblk % group == 0
    ngroups = nblk // group
    # 16-bit matmul dtypes are cast host-side: x/wt ship pre-cast, halving
    # input DMA bytes and skipping the on-chip rounding pass. f32r still
    # needs an on-chip DVE rounding producer.
    host_cast = mybir.dt.size(mm_dtype) == 2
    ship_dtype = mm_dtype if host_cast else F32
    if halo:
        assert host_cast, "halo path assumes 16-bit host-cast shipping"
        # input rows shipped once, row-outer
        x = nc.dram_tensor(
            "x", [XIN_ROWS, C, W], ship_dtype, kind="ExternalInput"
        ).ap()
    else:
        x = nc.dram_tensor(
            "x", [ngroups, 128, group * W], ship_dtype, kind="ExternalInput"
        ).ap()
    wt = nc.dram_tensor(
        "wt", [3, 128, 96], ship_dtype, kind="ExternalInput"
    ).ap()
    out_dtype = mybir.dt.float16 if out16 else F32
    y = nc.dram_tensor(
        "y", [C, R, nblk, WOUT], out_dtype, kind="ExternalOutput"
    ).ap()
    round_on_chip = (not host_cast) and mm_dtype != F32

    # Pre-TileContext prefetch: the TC preamble (engine barrier + sem/reg
    # init) costs ~7us during which the DMA queues would sit idle. Issue the
    # weights and the first two windows BEFORE entering TC, with manual
    # completion semaphores, so their transfers overlap the preamble and the
    # first matmul starts ~2.5us earlier. Only 3 issues (~0.6us each on the
    # issuing engine) so the entry barrier slips little.
    _PREGATE.clear()
    pre_sem = wsem = None
    w0bufs = None
    wall_raw = None
    # Disabled: bass places the TileContext preamble block first regardless
    # of emission order, so the "pre-TC" DMAs execute after the entry
    # barrier anyway — no overlap gained (measured 89.5us vs 85.4us).
    if False and not round_on_chip and not halo:
        pre_sem = nc.alloc_semaphore("w0pre")
        wsem = nc.alloc_semaphore("wtpre")
        nc.sync.sem_clear(pre_sem)
        nc.scalar.sem_clear(wsem)
        wall_raw = nc.alloc_sbuf_tensor("wallbuf", [128, 3, 96], mm_dtype)
        nc.scalar.dma_start(
            wall_raw.ap(), wt.rearrange("d p m -> p d m")
        ).then_inc(wsem, 16)
        w0bufs = []
        for w in range(2):
            h = nc.alloc_sbuf_tensor(f"w0buf{w}", [128, W], mm_dtype)
            nc.sync.dma_start(
                h.ap(), x[0][:, w * W : (w + 1) * W]
            ).then_inc(pre_sem, 16)
            w0bufs.append(h)

    with tile.TileContext(nc) as tc:
        with (
            tc.tile_pool(name="wpool", bufs=1) as wpool,
            tc.tile_pool(name="xpool", bufs=xbufs) as xpool,
            tc.tile_pool(name="winp", bufs=wbufs) as winp,
            tc.tile_pool(name="opool", bufs=obufs) as opool,
            tc.tile_pool(name="edgep", bufs=1) as edgep,
            tc.tile_pool(name="ppool", bufs=pbufs, space="PSUM") as ppool,
            tc.tile_pool(name="warmps", bufs=1, space="PSUM") as warmps,
        ):
            if not round_on_chip:
                # PE pre-warm: the PE is HAM-throttled to 1.2 GHz until it
                # has been busy ~3.4us, and it would otherwise sit idle from
                # the end of the preamble (~7us) until the first window DMA
                # lands (~11us). Run 8 dummy N=512 matmuls on a zeroed
                # scratch tile during that window so the real matmuls start
                # at the warm 216ns cadence instead of ~427ns.
                wsb_l = edgep.tile([128, 96], mm_dtype, tag="warmL", name="warmL")
                wsb_r = edgep.tile([128, 512], mm_dtype, tag="warmR", name="warmR")
                nc.vector.memset(wsb_l[:], 0.0)
                nc.vector.memset(wsb_r[:], 0.0)
                for i in range(8):
                    wps = warmps.tile(
                        [96, 512], F32, tag="wps", name=f"wps{i}"
                    )
                    nc.tensor.matmul(
                        wps[:], wsb_l[:], wsb_r[:], start=True, stop=True
                    )

            if round_on_chip:
                wts = []
                for dx in range(3):
                    ws = wpool.tile([128, 96], F32, tag=f"ws{dx}", name=f"ws{dx}")
                    nc.sync.dma_start(ws[:], wt[dx])
                    wtile = wpool.tile(
                        [128, 96], mm_dtype, tag=f"w{dx}", name=f"w{dx}"
                    )
                    nc.vector.tensor_copy(wtile[:], ws[:])
                    wts.append(wtile[:])
            elif wall_raw is not None:
                # weights were prefetched pre-TC into a raw sbuf buffer
                wts = [wall_raw.ap()[:, dx, :] for dx in range(3)]
            else:
                # single DMA for all 3 stationary operands, issued on the
                # (initially idle) scalar queue so the first input DMA owns
                # the sync queue from the start
                wall = wpool.tile([128, 3, 96], mm_dtype, tag="wall", name="wall")
                nc.scalar.dma_start(wall[:], wt.rearrange("d p m -> p d m"))
                wts = [wall[:, dx, :] for dx in range(3)]

            def evict(dst, src, on_vector):
                if on_vector:
                    if bias_sum == 0.0:
                        nc.vector.tensor_copy(dst, src)
                    else:
                        nc.vector.tensor_scalar_add(dst, src, float(bias_sum))
                else:
                    if bias_sum == 0.0:
                        nc.scalar.copy(dst, src)
                    else:
                        nc.scalar.add(dst, src, float(bias_sum))

            def window_mms(uid, w, rhs_win):
                psums = [
                    ppool.tile(
                        [96, 512], F32, tag="ps", name=f"ps_{uid}_{w}_{i}"
                    )
                    for i in range(len(CHUNKS))
                ]
                for dx in range(3):
                    for ic, (x0, n) in enumerate(CHUNKS):
                        nc.tensor.matmul(
                            psums[ic][:, :n],
                            wts[dx],
                            rhs_win[:, x0 + dx : x0 + dx + n],
                            start=(dx == 0),
                            stop=(dx == 2),
                        )
                return psums

            def sweep(prefix):
                prev_win = None
                for g in range(ngroups):
                    uid = f"{prefix}_{g}"
                    first, last = g == 0, g == ngroups - 1
                    if round_on_chip:
                        stage = xpool.tile(
                            [128, group, W], F32, tag="stage", name=f"st{uid}"
                        )
                        nc.sync.dma_start(
                            stage[:],
                            x[g].rearrange("p (w c) -> p w c", w=group),
                        )
                        win = winp.tile(
                            [128, group, W], mm_dtype, tag="win",
                            name=f"win{uid}",
                        )
                        nc.vector.tensor_copy(win[:], stage[:])
                        wins = [win[:, w, :] for w in range(group)]
                    elif halo:
                        # halo path (k-major partitions p = k*16+ci), fully
                        # per-window: the first window of the sweep fetches
                        # all 8 rows ([128, W] straight from row-major HBM);
                        # every later window fetches only its 6 fresh rows
                        # k=2..7 (partitions 32..127) and carries the 2 halo
                        # rows (partitions 0..31) from the previous window's
                        # k=6,7 (partitions 96..127) by one small DVE copy.
                        # Each input row is read from HBM exactly once
                        # (9.55 MB vs 12.6 preduped), and each copy waits
                        # only on a ~0.3MB DMA that completes well ahead, so
                        # the in-order DVE stream never stalls on them.
                        wins = []
                        for w in range(group):
                            b = g * group + w  # global window index
                            winw = winp.tile(
                                [128, W], mm_dtype, tag="win",
                                name=f"win{uid}_{w}",
                            )
                            if b == 0:
                                nc.sync.dma_start(
                                    winw[:],
                                    x[0:8].rearrange("k c x -> (k c) x"),
                                )
                            else:
                                nc.sync.dma_start(
                                    winw[32:128, :],
                                    x[6 * b + 2 : 6 * b + 8].rearrange(
                                        "k c x -> (k c) x"
                                    ),
                                )
                                nc.vector.tensor_copy(
                                    winw[0:32, :], prev_win[96:128, :]
                                )
                            wins.append(winw[:])
                            prev_win = winw
                    elif first:
                        # group 0: windows 0,1 were prefetched pre-TC (raw
                        # buffers, matmuls gate on pre_sem); windows 2,3 load
                        # per-window tiles here, on the scalar queue (idle
                        # during the ramp) so they overlap windows 0,1's use
                        wins = []
                        for w in range(group):
                            if w0bufs is not None and w < 2:
                                wins.append(w0bufs[w].ap())
                                continue
                            winw = edgep.tile(
                                [128, W], mm_dtype, tag=f"win0_{w}",
                                name=f"win{uid}_{w}",
                            )
                            eng = nc.sync if w < 2 else nc.scalar
                            eng.dma_start(
                                winw[:], x[g][:, w * W : (w + 1) * W]
                            )
                            wins.append(winw[:])
                    else:
                        # predup path: one contiguous DMA loads `group`
                        # preduped 8-row windows (p = ci*8+k). Splitting this
                        # DMA (per-window or w0+rest) was tried and is ~1.4us
                        # slower: extra DMAs pressure the 8 HWDGE completion
                        # semaphore lanes more than the boundary stall costs.
                        win = winp.tile(
                            [128, group, W], mm_dtype, tag="win",
                            name=f"win{uid}",
                        )
                        nc.sync.dma_start(
                            win[:],
                            x[g].rearrange("p (w c) -> p w c", w=group),
                        )
                        wins = [win[:, w, :] for w in range(group)]

                    # The last group stores per-window so the final output
                    # DMA is 1/4 the size (shorter drain tail).
                    if last:
                        for w in range(group):
                            otw = edgep.tile(
                                [96, W], out_dtype, tag=f"ow{w}",
                                name=f"o_{uid}_{w}",
                            )
                            psums = window_mms(uid, w, wins[w])
                            for ic, (x0, n) in enumerate(CHUNKS):
                                on_vec = split_evict and (
                                    ic == 1 or (ic == 2 and w % 2 == 1)
                                )
                                if w == group - 1 and ic == 2:
                                    on_vec = False  # scalar is free sooner
                                evict(
                                    otw[:, x0 : x0 + n],
                                    psums[ic][:, :n],
                                    on_vec,
                                )
                            if w == group - 1:
                                # final window: two half-DMAs so the second
                                # (issued right after the last eviction) only
                                # drains 0.1MB
                                nc.scalar.dma_start(
                                    y[:, :, g * group + w, 0:1024],
                                    otw[:96, 0:1024],
                                )
                                nc.scalar.dma_start(
                                    y[:, :, g * group + w, 1024:WOUT],
                                    otw[:96, 1024:WOUT],
                                )
                            else:
                                nc.scalar.dma_start(
                                    y[:, :, g * group + w, :],
                                    otw[:96, 0:WOUT],
                                )
                    else:
                        ot = opool.tile(
                            [96, group, W], out_dtype, tag="o", name=f"o_{uid}"
                        )
                        for w in range(group):
                            if first and pre_sem is not None and w < 2:
                                # gate on the pre-TC prefetch completions;
                                # emitted as >=0 for the scheduler sim and
                                # patched to the real value at lowering
                                if w == 0:
                                    iw = nc.tensor.wait_ge(wsem, 0)
                                    _PREGATE.append(
                                        (iw.ins.sync_info.on_wait[0], 16)
                                    )
                                ip = nc.tensor.wait_ge(pre_sem, 0)
                                _PREGATE.append(
                                    (ip.ins.sync_info.on_wait[0], 16 * (w + 1))
                                )
                            psums = window_mms(uid, w, wins[w])
                            # PSUM -> SBUF eviction (+ bias). Scalar alone is
                            # ~66us for the sweep; splitting chunks between
                            # the scalar and vector engines halves that.
                            for ic, (x0, n) in enumerate(CHUNKS):
                                evict(
                                    ot[:, w, x0 : x0 + n],
                                    psums[ic][:, :n],
                                    split_evict
                                    and (ic == 1 or (ic == 2 and w % 2 == 1)),
                                )
                        # one DMA stores `group` blocks of 6 output rows into
                        # the [C, R, nblk, WOUT] device layout; (blk, x)
                        # merge keeps the dest AP 3-dim. scalar-engine HWDGE
                        # queue keeps outputs off the input queue.
                        nc.scalar.dma_start(
                            y[:, :, g * group : (g + 1) * group, :].rearrange(
                                "c r b x -> c r (b x)"
                            ),
                            ot[:96, :, 0:WOUT],
                        )

            if hw_loop:
                with tc.For_i(
                    0, hw_loop, 1, hint_engines=(mybir.EngineType.PE,)
                ):
                    sweep("L")
            else:
                for rep in range(repeat):
                    sweep(str(rep))
    return nc


def prepare(X: np.ndarray, weight: np.ndarray, bias: np.ndarray):
    """Build the per-core program and input maps shared by kernel() and any
    external profiler. Returns (nc, in_maps, starts)."""
    X = np.ascontiguousarray(np.asarray(X, dtype=np.float32))
    weight = np.asarray(weight, dtype=np.float32)
    bias = np.asarray(bias, dtype=np.float32)

    _patch_tile_drain()
    import os

    mm_dtype = {
        "f32": F32,
        "f32r": F32R,
        "f16": mybir.dt.float16,
        "bf16": mybir.dt.bfloat16,
    }[os.environ.get("CONV_MM_DTYPE", "f16")]
    halo = os.environ.get("CONV_HALO", "0") == "1"
    group = 4
    host_cast = mybir.dt.size(mm_dtype) == 2
    ship = mybir.dt.np(mm_dtype) if host_cast else np.float32
    halo = halo and host_cast
    wbufs = 8 if halo else (6 if host_cast else 2)
    lhsT = build_lhsT(weight, k_major=halo)
    nc = build_program(
        float(bias.sum()), mm_dtype, group=group, wbufs=wbufs, pbufs=7,
        halo=halo,
    )

    starts = [min(c * ROWS_PER_CORE, H - XIN_ROWS) for c in range(NCORES)]
    in_maps = []
    for s in starts:
        Xs = X[:, s : s + XIN_ROWS, :]
        xr = (
            shard_rows(Xs, ship) if halo else shard_windows(Xs, group, ship)
        )
        in_maps.append({"x": xr, "wt": lhsT.astype(ship)})
    return nc, in_maps, starts


def kernel(X: np.ndarray, weight: np.ndarray, bias: np.ndarray) -> np.ndarray:
    nc, in_maps, starts = prepare(X, weight, bias)
    # the device occasionally faults transiently (NRT_EXEC_UNIT_UNRECOVERABLE)
    # -- retry a couple of times before giving up
    last_err = None
    for _ in range(3):
        try:
            res = run_bass_kernel_spmd(
                nc, in_maps, core_ids=list(range(NCORES))
            )
            break
        except Exception as e:  # noqa: BLE001
            last_err = e
    else:
        raise last_err

    out = np.empty((C, HOUT, WOUT), np.float32)
    for c in range(NCORES):
        yc = res.results[c]["y"].astype(np.float32)  # [C, R, NBLK, WOUT]
        out[:, starts[c] : starts[c] + ROWS_PER_CORE, :] = (
            yc.transpose(0, 2, 1, 3).reshape(C, ROWS_PER_CORE, WOUT)
        )
    return out



# revision 59
# speedup vs baseline: 1.1025x; 1.1025x over previous
"""Trainium2 Bass kernel for 16->16 channel 3x3 VALID conv on [16,1536,1536].

out[co, y, x] = sum_{ci,dy,dx} W[co,ci,dy,dx] * X[ci, y+dy, x+dx] + sum(bias)

Strategy (8-core data parallel over H, halo of 2 rows):
  Each core computes 192 output rows from a 194-row input shard, in 32 blocks
  of R=6 output rows. Per block, an SBUF "window" of 8 input rows x 16
  channels is laid out as [128, 1536] with partition p = ci*8 + k (k = row in
  window). The conv becomes 3 accumulating matmuls (one per kernel column dx)
  whose dx shift is a free-dim offset on the moving operand:
     psum[m=(co*6+r), x] += lhsT_dx[p, m] * window[p, x+dx]
  with block-Toeplitz weights lhsT_dx[ci*8+k, co*6+r] = W[co,ci,k-r,dx]
  (zero outside 0 <= k-r <= 2), precomputed on host from the 9KB weight.
  Contraction K=128, M=96 -> 3 column-streams per 6 output pixels; measured
  PE cadence is the N/2.4GHz streaming floor (216ns per 512-col matmul,
  LDWEIGHTS fully hidden), ~62us/core for the sweep.

  Matmuls run in float16 (1 col/cycle; fp32 would be 4x slower). Inputs are
  cast to f16 on host (halves input DMA bytes); outputs are stored f16 and
  upcast on host (halves output DMA bytes; tolerance is 2e-2, f16 adds
  ~4e-4). HBM/core: 12.6 MB in + 9.4 MB out = 22 MB ~= 61.5us at 358 GB/s,
  right at the PE floor ("ridge").

  Inputs arrive as host-preduped window groups of 4 (one contiguous DMA per
  group on the sync-engine HWDGE queue; the 33% halo duplication is cheaper
  than any on-chip halo-carry variant - those all lose to in-order engine /
  DMA-completion-semaphore serialization, measured 96-148us). Group 0 loads
  per-window tiles so the first matmul starts at ~11us. PSUM eviction (+bias,
  f32->f16 cast) is split between the scalar engine (chunk 0, and chunk 2 on
  even windows) and the vector engine (chunk 1, chunk 2 on odd windows).
  Outputs leave as one grouped DMA per 4 windows on the scalar-engine HWDGE
  queue; the last group stores per-window so the final drain is 1/4 the size.
  Weights load via the scalar queue so the first input DMA owns sync.
"""

import numpy as np

import concourse.bass as bass
import concourse.mybir as mybir
import concourse.tile as tile
from concourse.bass_utils import run_bass_kernel_spmd

C = 16
H = 1536
W = 1536
HOUT = H - 2
WOUT = W - 2
NCORES = 8
ROWS_PER_CORE = 192  # output rows computed per core
R = 6                # output rows per block
WIN = R + 2          # input rows per window
NBLK = ROWS_PER_CORE // R
XIN_ROWS = ROWS_PER_CORE + 2  # input rows per shard
CHUNKS = [(0, 512), (512, 512), (1024, WOUT - 1024)]
F32 = mybir.dt.float32
F32R = mybir.dt.float32r

_drain_patched = False
# (SyncWait, value) pairs whose wait_value is injected after the Tile
# scheduler's CoreSim pass (the sim cannot see pre-TileContext DMA
# completions, so the waits are emitted as >=0 and patched to the real
# thresholds just before lowering)
_PREGATE = []


def _patch_tile_drain():
    """This container's walrus accepts only ONE sync-wait per lowered
    instruction (CTRL drains, S3_LW weight loads, ...). Tile freely attaches
    several. Split the extras onto single-wait nops placed just before the
    instruction on the same engine (identical blocking semantics)."""
    global _drain_patched
    if _drain_patched:
        return
    _drain_patched = True
    from concourse.tile import ScopedClock

    def _split_multi_waits(ordered):
        for bb_name, insts in ordered.items():
            out = []
            for inst in insts:
                si = getattr(inst, "sync_info", None)
                if (
                    si is not None
                    and si.on_wait is not None
                    and len(si.on_wait) > 1
                    and type(inst).__module__ == "bass_rust"
                ):
                    waits = list(si.on_wait)
                    for i, w in enumerate(waits[:-1]):
                        out.append(
                            mybir.InstNoOp(
                                name=f"{inst.name}ws{i}",
                                engine=inst.engine,
                                bass_nofuse=True,
                                sync_info=mybir.SyncInfo(
                                    on_wait=[w], on_update=[]
                                ),
                            )
                        )
                    inst.sync_info = mybir.SyncInfo(
                        on_wait=[waits[-1]],
                        on_update=list(si.on_update or []),
                    )
                out.append(inst)
            ordered[bb_name] = out
        return ordered

    orig_lower = tile.TileContext._lower_ordered_insts

    def _lower_ordered_insts(self, ordered):
        for wobj, val in _PREGATE:
            wobj.wait_value = val
        _PREGATE.clear()
        return orig_lower(self, _split_multi_waits(ordered))

    tile.TileContext._lower_ordered_insts = _lower_ordered_insts

    def _drain_and_barrier(self, tick_clock, wait_clock):
        drain_inst = self.nc.sync.drain()
        wait_clock.add_sem_waits(
            drain_inst.ins, ScopedClock({None: tick_clock.global_clock})
        )
        si = drain_inst.ins.sync_info
        if si is not None and si.on_wait is not None and len(si.on_wait) > 1:
            waits = list(si.on_wait)
            drain_inst.ins.sync_info = mybir.SyncInfo(
                on_wait=[waits[0]], on_update=list(si.on_update or [])
            )
            for w in waits[1:]:
                n = self.nc.sync.nop(nofuse=True, hint="drain_wait_split")
                n.ins.sync_info = mybir.SyncInfo(on_wait=[w], on_update=[])
        self.nc.all_engine_barrier()
        assert self.sems is not None
        popped = self.nc._tile_sem_poison_stack.pop()
        assert popped is self._sem_poison
        self.nc.clear_and_free_semaphores(list(self.sems.allocated().values()))
        self.nc.all_engine_barrier()

    tile.TileContext._drain_and_barrier = _drain_and_barrier


def build_lhsT(weight: np.ndarray, k_major: bool = False) -> np.ndarray:
    """[C_out=16, C_in=16, 3, 3] -> [3, 128, 96] block-Toeplitz stationary
    operands, one per kernel column dx.
    ci-major: lhsT[dx, ci*8+k, co*6+r] = weight[co, ci, k-r, dx] (0<=k-r<=2)
    k-major:  lhsT[dx, k*16+ci, co*6+r] = same (used by the halo path, where
    window rows k must be partition-contiguous for the DVE halo copies)."""
    lhsT = np.zeros((3, 128, 96), np.float32)
    ci = np.arange(C)
    for dx in range(3):
        for dy in range(3):
            for r in range(R):
                k = r + dy
                p = (k * C + ci) if k_major else (ci * WIN + k)
                lhsT[dx, p[:, None], (np.arange(C) * R + r)[None, :]] = (
                    weight[:, :, dy, dx].T
                )
    return lhsT


def shard_rows(Xs: np.ndarray, dtype=np.float16):
    """Halo-path host prep for one core's shard [C, XIN_ROWS, W]: rows
    shipped once, row-outer [XIN_ROWS, C, W] so (k c) merges contiguously
    in the window DMAs."""
    return np.ascontiguousarray(Xs.transpose(1, 0, 2).astype(dtype))


def shard_windows(
    Xs: np.ndarray, group: int, dtype=np.float32
) -> np.ndarray:
    """Host-side window predup for one core's shard [C, XIN_ROWS, W] ->
    [ngroups, 128, group*W] where out[g, ci*8+k, w*W+x] =
    Xs[ci, 6*(group*g+w)+k, x]. dma_start carries a large fixed overhead
    here, so shipping the 33% halo duplication in exchange for one
    contiguous multi-MB DMA per group is a clear win. For 16-bit matmul
    dtypes the cast happens here too, halving the DMA bytes."""
    ngroups = NBLK // group
    rows = (
        R * group * np.arange(ngroups)[:, None, None]
        + R * np.arange(group)[None, :, None]
        + np.arange(WIN)[None, None, :]
    )  # [g, w, k]
    arr = Xs[:, rows, :].astype(dtype)  # [C, g, w, k, W]
    arr = arr.transpose(1, 0, 3, 2, 4)  # [g, C, k, w, W]
    return np.ascontiguousarray(
        arr.reshape(ngroups, 128, group * W)
    )


def build_program(
    bias_sum: float,
    mm_dtype=F32R,
    nblk=NBLK,
    group=4,
    xbufs=3,
    wbufs=2,
    obufs=4,
    pbufs=6,
    repeat=1,
    hw_loop=0,
    out16=True,
    split_evict=True,
    halo=True,
):
    """One core's program: see module docstring. dma_start carries a large
    fixed overhead in this environment, so inputs arrive as host-preduped
    window groups (one contiguous DMA per `group` blocks) and outputs leave
    as one grouped DMA into a [C, R, nblk, WOUT] device layout the host
    re-transposes. `repeat` (python-unrolled) and `hw_loop` (tc.For_i)
    re-run the whole block sweep, for timing amplification only."""
    nc = bass.Bass("TRN2", target_bir_lowering=False, debug=False)
    assert nblk % group == 0
    ngroups = nblk // group
    # 16-bit matmul dtypes are cast host-side: x/wt ship pre-cast, halving
    # input DMA bytes and skipping the on-chip rounding pass. f32r still
    # needs an on-chip DVE rounding producer.
    host_cast = mybir.dt.size(mm_dtype) == 2
    ship_dtype = mm_dtype if host_cast else F32
    if halo:
        assert host_cast, "halo path assumes 16-bit host-cast shipping"
        # input rows shipped once, row-outer
        x = nc.dram_tensor(
            "x", [XIN_ROWS, C, W], ship_dtype, kind="ExternalInput"
        ).ap()
    else:
        x = nc.dram_tensor(
            "x", [ngroups, 128, group * W], ship_dtype, kind="ExternalInput"
        ).ap()
    wt = nc.dram_tensor(
        "wt", [3, 128, 96], ship_dtype, kind="ExternalInput"
    ).ap()
    out_dtype = mybir.dt.float16 if out16 else F32
    y = nc.dram_tensor(
        "y", [C, R, nblk, WOUT], out_dtype, kind="ExternalOutput"
    ).ap()
    round_on_chip = (not host_cast) and mm_dtype != F32

    # Pre-TileContext prefetch: the TC preamble (engine barrier + sem/reg
    # init) costs ~7us during which the DMA queues would sit idle. Issue the
    # weights and the first two windows BEFORE entering TC, with manual
    # completion semaphores, so their transfers overlap the preamble and the
    # first matmul starts ~2.5us earlier. Only 3 issues (~0.6us each on the
    # issuing engine) so the entry barrier slips little.
    _PREGATE.clear()
    pre_sem = wsem = None
    w0bufs = None
    wall_raw = None
    # Disabled: bass places the TileContext preamble block first regardless
    # of emission order, so the "pre-TC" DMAs execute after the entry
    # barrier anyway — no overlap gained (measured 89.5us vs 85.4us).
    if False and not round_on_chip and not halo:
        pre_sem = nc.alloc_semaphore("w0pre")
        wsem = nc.alloc_semaphore("wtpre")
        nc.sync.sem_clear(pre_sem)
        nc.scalar.sem_clear(wsem)
        wall_raw = nc.alloc_sbuf_tensor("wallbuf", [128, 3, 96], mm_dtype)
        nc.scalar.dma_start(
            wall_raw.ap(), wt.rearrange("d p m -> p d m")
        ).then_inc(wsem, 16)
        w0bufs = []
        for w in range(2):
            h = nc.alloc_sbuf_tensor(f"w0buf{w}", [128, W], mm_dtype)
            nc.sync.dma_start(
                h.ap(), x[0][:, w * W : (w + 1) * W]
            ).then_inc(pre_sem, 16)
            w0bufs.append(h)

    with tile.TileContext(nc) as tc:
        with (
            tc.tile_pool(name="wpool", bufs=1) as wpool,
            tc.tile_pool(name="xpool", bufs=xbufs) as xpool,
            tc.tile_pool(name="winp", bufs=wbufs) as winp,
            tc.tile_pool(name="opool", bufs=obufs) as opool,
            tc.tile_pool(name="edgep", bufs=1) as edgep,
            tc.tile_pool(name="ppool", bufs=pbufs, space="PSUM") as ppool,
        ):
            # (PE pre-warm via dummy matmuls during the first-window DMA
            # wait was tried here: a fused 8-MM accumulation fails NEFF
            # load, and independent warm MMs + pbufs=7 measured 93.1us vs
            # 84.8 — reverted.)
            if round_on_chip:
                wts = []
                for dx in range(3):
                    ws = wpool.tile([128, 96], F32, tag=f"ws{dx}", name=f"ws{dx}")
                    nc.sync.dma_start(ws[:], wt[dx])
                    wtile = wpool.tile(
                        [128, 96], mm_dtype, tag=f"w{dx}", name=f"w{dx}"
                    )
                    nc.vector.tensor_copy(wtile[:], ws[:])
                    wts.append(wtile[:])
            elif wall_raw is not None:
                # weights were prefetched pre-TC into a raw sbuf buffer
                wts = [wall_raw.ap()[:, dx, :] for dx in range(3)]
            else:
                # single DMA for all 3 stationary operands, issued on the
                # (initially idle) scalar queue so the first input DMA owns
                # the sync queue from the start
                wall = wpool.tile([128, 3, 96], mm_dtype, tag="wall", name="wall")
                nc.scalar.dma_start(wall[:], wt.rearrange("d p m -> p d m"))
                wts = [wall[:, dx, :] for dx in range(3)]

            def evict(dst, src, on_vector):
                if on_vector:
                    if bias_sum == 0.0:
                        nc.vector.tensor_copy(dst, src)
                    else:
                        nc.vector.tensor_scalar_add(dst, src, float(bias_sum))
                else:
                    if bias_sum == 0.0:
                        nc.scalar.copy(dst, src)
                    else:
                        nc.scalar.add(dst, src, float(bias_sum))

            def window_mms(uid, w, rhs_win):
                psums = [
                    ppool.tile(
                        [96, 512], F32, tag="ps", name=f"ps_{uid}_{w}_{i}"
                    )
                    for i in range(len(CHUNKS))
                ]
                for dx in range(3):
                    for ic, (x0, n) in enumerate(CHUNKS):
                        nc.tensor.matmul(
                            psums[ic][:, :n],
                            wts[dx],
                            rhs_win[:, x0 + dx : x0 + dx + n],
                            start=(dx == 0),
                            stop=(dx == 2),
                        )
                return psums

            def sweep(prefix):
                prev_win = None
                for g in range(ngroups):
                    uid = f"{prefix}_{g}"
                    first, last = g == 0, g == ngroups - 1
                    if round_on_chip:
                        stage = xpool.tile(
                            [128, group, W], F32, tag="stage", name=f"st{uid}"
                        )
                        nc.sync.dma_start(
                            stage[:],
                            x[g].rearrange("p (w c) -> p w c", w=group),
                        )
                        win = winp.tile(
                            [128, group, W], mm_dtype, tag="win",
                            name=f"win{uid}",
                        )
                        nc.vector.tensor_copy(win[:], stage[:])
                        wins = [win[:, w, :] for w in range(group)]
                    elif halo:
                        # halo path (k-major partitions p = k*16+ci), fully
                        # per-window: the first window of the sweep fetches
                        # all 8 rows ([128, W] straight from row-major HBM);
                        # every later window fetches only its 6 fresh rows
                        # k=2..7 (partitions 32..127) and carries the 2 halo
                        # rows (partitions 0..31) from the previous window's
                        # k=6,7 (partitions 96..127) by one small DVE copy.
                        # Each input row is read from HBM exactly once
                        # (9.55 MB vs 12.6 preduped), and each copy waits
                        # only on a ~0.3MB DMA that completes well ahead, so
                        # the in-order DVE stream never stalls on them.
                        wins = []
                        for w in range(group):
                            b = g * group + w  # global window index
                            winw = winp.tile(
                                [128, W], mm_dtype, tag="win",
                                name=f"win{uid}_{w}",
                            )
                            if b == 0:
                                nc.sync.dma_start(
                                    winw[:],
                                    x[0:8].rearrange("k c x -> (k c) x"),
                                )
                            else:
                                nc.sync.dma_start(
                                    winw[32:128, :],
                                    x[6 * b + 2 : 6 * b + 8].rearrange(
                                        "k c x -> (k c) x"
                                    ),
                                )
                                nc.vector.tensor_copy(
                                    winw[0:32, :], prev_win[96:128, :]
                                )
                            wins.append(winw[:])
                            prev_win = winw
                    elif first:
                        # group 0: windows 0,1 were prefetched pre-TC (raw
                        # buffers, matmuls gate on pre_sem); windows 2,3 load
                        # per-window tiles here, on the scalar queue (idle
                        # during the ramp) so they overlap windows 0,1's use
                        wins = []
                        for w in range(group):
                            if w0bufs is not None and w < 2:
                                wins.append(w0bufs[w].ap())
                                continue
                            winw = edgep.tile(
                                [128, W], mm_dtype, tag=f"win0_{w}",
                                name=f"win{uid}_{w}",
                            )
                            eng = nc.sync if w < 2 else nc.scalar
                            eng.dma_start(
                                winw[:], x[g][:, w * W : (w + 1) * W]
                            )
                            wins.append(winw[:])
                    else:
                        # predup path: one contiguous DMA loads `group`
                        # preduped 8-row windows (p = ci*8+k). Splitting this
                        # DMA (per-window or w0+rest) was tried and is ~1.4us
                        # slower: extra DMAs pressure the 8 HWDGE completion
                        # semaphore lanes more than the boundary stall costs.
                        win = winp.tile(
                            [128, group, W], mm_dtype, tag="win",
                            name=f"win{uid}",
                        )
                        nc.sync.dma_start(
                            win[:],
                            x[g].rearrange("p (w c) -> p w c", w=group),
                        )
                        wins = [win[:, w, :] for w in range(group)]

                    # The last group stores per-window so the final output
                    # DMA is 1/4 the size (shorter drain tail).
                    if last:
                        for w in range(group):
                            otw = edgep.tile(
                                [96, W], out_dtype, tag=f"ow{w}",
                                name=f"o_{uid}_{w}",
                            )
                            psums = window_mms(uid, w, wins[w])
                            for ic, (x0, n) in enumerate(CHUNKS):
                                on_vec = split_evict and (
                                    ic == 1 or (ic == 2 and w % 2 == 1)
                                )
                                if w == group - 1 and ic == 2:
                                    on_vec = False  # scalar is free sooner
                                evict(
                                    otw[:, x0 : x0 + n],
                                    psums[ic][:, :n],
                                    on_vec,
                                )
                            if w == group - 1:
                                # final window: two half-DMAs so the second
                                # (issued right after the last eviction) only
                                # drains 0.1MB
                                nc.scalar.dma_start(
                                    y[:, :, g * group + w, 0:1024],
                                    otw[:96, 0:1024],
                                )
                                nc.scalar.dma_start(
                                    y[:, :, g * group + w, 1024:WOUT],
                                    otw[:96, 1024:WOUT],
                                )
                            else:
                                nc.scalar.dma_start(
                                    y[:, :, g * group + w, :],
                                    otw[:96, 0:WOUT],
                                )
                    else:
                        ot = opool.tile(
                            [96, group, W], out_dtype, tag="o", name=f"o_{uid}"
                        )
                        for w in range(group):
                            if first and pre_sem is not None and w < 2:
                                # gate on the pre-TC prefetch completions;
                                # emitted as >=0 for the scheduler sim and
                                # patched to the real value at lowering
                                if w == 0:
                                    iw = nc.tensor.wait_ge(wsem, 0)
                                    _PREGATE.append(
                                        (iw.ins.sync_info.on_wait[0], 16)
                                    )
                                ip = nc.tensor.wait_ge(pre_sem, 0)
                                _PREGATE.append(
                                    (ip.ins.sync_info.on_wait[0], 16 * (w + 1))
                                )
                            psums = window_mms(uid, w, wins[w])
                            # PSUM -> SBUF eviction (+ bias). Scalar alone is
                            # ~66us for the sweep; splitting chunks between
                            # the scalar and vector engines halves that.
                            for ic, (x0, n) in enumerate(CHUNKS):
                                evict(
                                    ot[:, w, x0 : x0 + n],
                                    psums[ic][:, :n],
                                    split_evict
                                    and (ic == 1 or (ic == 2 and w % 2 == 1)),
                                )
                        # one DMA stores `group` blocks of 6 output rows into
                        # the [C, R, nblk, WOUT] device layout; (blk, x)
                        # merge keeps the dest AP 3-dim. scalar-engine HWDGE
                        # queue keeps outputs off the input queue.
                        nc.scalar.dma_start(
                            y[:, :, g * group : (g + 1) * group, :].rearrange(
                                "c r b x -> c r (b x)"
                            ),
                            ot[:96, :, 0:WOUT],
                        )

            if hw_loop:
                with tc.For_i(
                    0, hw_loop, 1, hint_engines=(mybir.EngineType.PE,)
                ):
                    sweep("L")
            else:
                for rep in range(repeat):
                    sweep(str(rep))
    return nc


def prepare(X: np.ndarray, weight: np.ndarray, bias: np.ndarray):
    """Build the per-core program and input maps shared by kernel() and any
    external profiler. Returns (nc, in_maps, starts)."""
    X = np.ascontiguousarray(np.asarray(X, dtype=np.float32))
    weight = np.asarray(weight, dtype=np.float32)
    bias = np.asarray(bias, dtype=np.float32)

    _patch_tile_drain()
    import os

    mm_dtype = {
        "f32": F32,
        "f32r": F32R,
        "f16": mybir.dt.float16,
        "bf16": mybir.dt.bfloat16,
    }[os.environ.get("CONV_MM_DTYPE", "f16")]
    halo = os.environ.get("CONV_HALO", "0") == "1"
    group = 4
    host_cast = mybir.dt.size(mm_dtype) == 2
    ship = mybir.dt.np(mm_dtype) if host_cast else np.float32
    halo = halo and host_cast
    wbufs = 8 if halo else (6 if host_cast else 2)
    lhsT = build_lhsT(weight, k_major=halo)
    nc = build_program(
        float(bias.sum()), mm_dtype, group=group, wbufs=wbufs, pbufs=8,
        halo=halo,
    )

    starts = [min(c * ROWS_PER_CORE, H - XIN_ROWS) for c in range(NCORES)]
    in_maps = []
    for s in starts:
        Xs = X[:, s : s + XIN_ROWS, :]
        xr = (
            shard_rows(Xs, ship) if halo else shard_windows(Xs, group, ship)
        )
        in_maps.append({"x": xr, "wt": lhsT.astype(ship)})
    return nc, in_maps, starts


def kernel(X: np.ndarray, weight: np.ndarray, bias: np.ndarray) -> np.ndarray:
    nc, in_maps, starts = prepare(X, weight, bias)
    # the device occasionally faults transiently (NRT_EXEC_UNIT_UNRECOVERABLE)
    # -- retry a couple of times before giving up
    last_err = None
    for _ in range(3):
        try:
            res = run_bass_kernel_spmd(
                nc, in_maps, core_ids=list(range(NCORES))
            )
            break
        except Exception as e:  # noqa: BLE001
            last_err = e
    else:
        raise last_err

    out = np.empty((C, HOUT, WOUT), np.float32)
    for c in range(NCORES):
        yc = res.results[c]["y"].astype(np.float32)  # [C, R, NBLK, WOUT]
        out[:, starts[c] : starts[c] + ROWS_PER_CORE, :] = (
            yc.transpose(0, 2, 1, 3).reshape(C, ROWS_PER_CORE, WOUT)
        )
    return out

